# revision 1
# baseline (speedup 1.0000x reference)
"""DialogueGCN forward pass as a distributed Bass/Tile kernel on 8 TRN2 NeuronCores.

Math (reference): Bahdanau attention over utterance pairs -> per-edge softmax
weights; RGCN with per-relation weights W_rel[etype] + root term; GraphConv.

Key structural facts exploited:
  * etype = 2*(sp[i]*48 + sp[j]) + (i>=j) with speaker in {0,1} -> at most 8
    distinct relation types out of R=4608 are live. Only those 8 slices of the
    1.2GB W_rel are ever touched (host routes them to the devices).
  * The graph is fully connected, so the GraphConv neighbor sum is the same
    vector for every node: sum_i h_i.
  * agg = sum_r (attn*mask_r)^T (x @ W_r): 8 masked matmuls instead of a
    2304-edge gather/scatter.

Sharding: the RGCN/GraphConv hidden dim H=256 is split 8 ways (32 columns per
core); every core computes the full [48,48] attention (cheap, avoids a second
collective), its h-slice of the RGCN, then one AllGather of the [32,48] slices
rebuilds h^T [256,48] on every core, and each core finishes its g-slice of the
GraphConv output. Host concatenates the 8 [32,48] outputs and transposes.
"""
import numpy as np

L = 48
D = 256
H = 256
G = 256
A = 128
N_CORES = 8
HS = H // N_CORES  # 32 columns of h (and of the output) per core
NREL = 8

_compiled = None


def _emit_body(nc, mybir, pool, psum, dram, d, rep, collective, n_cores):
    """Emit one full forward pass. `d` maps dram-tensor names to handles."""
    dt = mybir.dt.float32
    u = f"_{rep}"

    # ---- three packed input DMAs, on three different engine queues ----
    apk = pool.tile([128, 2, 304], mybir.dt.float32r, name=f"apk{u}", tag="apk")
    rpk = pool.tile([128, 2, 352], mybir.dt.float32r, name=f"rpk{u}", tag="rpk")
    spk = pool.tile([128, 387], dt, name=f"spk{u}", tag="spk")
    for t in range(2):
        nc.sync.dma_start(apk[:, t, :], d["apack"].ap()[t])
        nc.gpsimd.dma_start(rpk[:, t, :], d["rpack"].ap()[t])
    nc.scalar.dma_start(spk[:], d["spack"].ap())

    def xt(t):
        return apk[:, t, 0:L]
    def wqs(t):
        return apk[:, t, L:L + A]
    def wks(t):
        return apk[:, t, L + A:L + 2 * A]
    def wr(t):
        return rpk[:, t, 0:NREL * HS]
    def wro(t):
        return rpk[:, t, NREL * HS:NREL * HS + HS]
    def wsl(t):
        return rpk[:, t, NREL * HS + HS:NREL * HS + 2 * HS]
    def wn(t):
        return rpk[:, t, NREL * HS + 2 * HS:NREL * HS + 3 * HS]
    vv = spk[:, 0:1]
    maskt = spk[0:L, 1:1 + NREL * L].rearrange("i (r j) -> i r j", r=NREL)
    brg = spk[0:HS, 1 + NREL * L:2 + NREL * L]
    bgc = spk[0:HS, 2 + NREL * L:3 + NREL * L]

    # f32r (TF32-like) matmul operands: 4x PE rate on wide outputs, ~1e-3
    # matmul precision -- well inside the output tolerance here.
    f32r = mybir.dt.float32r
    # ---- Bahdanau attention: scores[i,j] = v . tanh(qT[:,i]+kT[:,j]) ----
    # qT/kT stay in PSUM; the broadcast add reads them there directly
    qT_ps = psum.tile([128, L, 1], dt, name=f"qT_ps{u}", tag="qt_ps")
    for t in range(2):
        nc.tensor.matmul(qT_ps[:, :, 0], wqs(t).bitcast(dt), xt(t).bitcast(dt),
                         start=(t == 0), stop=(t == 1))
    kT_ps = psum.tile([128, 1, L], dt, name=f"kT_ps{u}", tag="kt_ps")
    for t in range(2):
        nc.tensor.matmul(kT_ps[:, 0, :], wks(t).bitcast(dt), xt(t).bitcast(dt),
                         start=(t == 0), stop=(t == 1))
    kTs = pool.tile([128, 1, L], dt, name=f"kTs{u}", tag="kTs")
    nc.vector.tensor_copy(kTs[:, 0, :], kT_ps[:, 0, :])
    # qT to SBUF too: an all-SBUF tensor_tensor runs the DVE 2x fp32 mode
    qTs = pool.tile([128, L, 1], dt, name=f"qTs{u}", tag="qTs")
    nc.scalar.copy(qTs[:, :, 0], qT_ps[:, :, 0])

    # RGCN matmuls that need only the input DMAs come first so PE is warm
    # before (and during) the attention chain.
    yall_ps = psum.tile([L, NREL * HS], dt, name=f"yall_ps{u}", tag="mm_ps")
    for t in range(2):
        nc.tensor.matmul(yall_ps[:], xt(t).bitcast(dt), wr(t).bitcast(dt),
                         start=(t == 0), stop=(t == 1))
    yall = pool.tile([L, NREL * HS], dt, name=f"yall{u}", tag="yall")
    nc.vector.tensor_copy(yall[:], yall_ps[:])
    h_ps = psum.tile([HS, L], dt, name=f"h_ps{u}", tag="mm_ps")
    for t in range(2):
        nc.tensor.matmul(h_ps[:], wro(t).bitcast(dt), xt(t).bitcast(dt),
                         start=(t == 0), stop=False)

    # broadcast add + tanh + v-matmul, in 4 chunks so DVE/ACT/PE pipeline
    NCH = 4
    CL = L // NCH  # 12 query rows per chunk = 576 floats
    bigT = pool.tile([128, L, L], dt, name=f"bigT{u}", tag="bigT")
    # tanh output + v are written as float32r so the score matmuls can run
    # the 4x-rate f32r PE path (producers must round to f32r per verifier)
    tanhT = pool.tile([128, L * L], f32r, name=f"tanhT{u}", tag="tanhT")
    vvr = pool.tile([128, 1], f32r, name=f"vvr{u}", tag="vvr")
    nc.vector.tensor_copy(vvr[:], vv)
    scores_ps = psum.tile([1, L * L], dt, name=f"scores_ps{u}", tag="attn_big")
    next_k = 0  # matmul outputs must stay inside one PSUM bank: 512-aligned
    for h in range(NCH):
        isl = slice(h * CL, (h + 1) * CL)
        csl = slice(h * CL * L, (h + 1) * CL * L)
        nc.vector.tensor_tensor(
            bigT[:, isl, :],
            qTs[:, isl, :].broadcast_to([128, CL, L]),
            kTs[:].broadcast_to([128, CL, L]),
            op=mybir.AluOpType.add,
        )
        nc.scalar.activation(tanhT[:, csl],
                             bigT[:, isl, :].rearrange("p i j -> p (i j)"),
                             mybir.ActivationFunctionType.Tanh)
        covered = (h + 1) * CL * L
        while next_k + 512 <= covered or (h == NCH - 1 and next_k < L * L):
            hi = min(next_k + 512, L * L)
            nc.tensor.matmul(scores_ps[:, next_k:hi], vvr[:], tanhT[:, next_k:hi],
                             start=True, stop=True)
            next_k = hi
    # PSUM -> SBUF [1, 2304]: single-partition copy, per-chunk, alternating
    # DVE/ACT so it pipelines behind the score matmuls. Each chunk's slice of
    # the DRAM bounce is written immediately after its copy (3 queues) so the
    # out-leg hides behind the copy pipeline.
    scores_row = pool.tile([1, L * L], dt, name=f"scores_row{u}", tag="scores_row")
    scores_dram = dram.tile([1, L * L], dt, name=f"scores_dram{u}", tag="scores_dram")
    out_engs = [nc.sync, nc.scalar, nc.sync, nc.scalar, nc.gpsimd]
    for ci, k in enumerate(range(0, L * L, 512)):
        hi = min(k + 512, L * L)
        if ci % 2 == 0:
            nc.vector.tensor_copy(scores_row[:, k:hi], scores_ps[:, k:hi])
        else:
            nc.scalar.copy(scores_row[:, k:hi], scores_ps[:, k:hi])
        out_engs[ci].dma_start(scores_dram[:, k:hi], scores_row[:, k:hi])
    # reshape [1, 2304] -> [48, 48] has to cross partitions: load back in two
    # row-halves (SBUF partition starts must be 32-aligned; float offset 1536
    # aligns with the 512-float chunks) and run the softmax exp per half.
    scores = pool.tile([L, L], dt, name=f"scores{u}", tag="scores")
    expS = pool.tile([L, L], dt, name=f"expS{u}", tag="expS")
    rowsum = pool.tile([L, 1], dt, name=f"rowsum{u}", tag="rowsum")
    sd_rows = scores_dram[:].rearrange("p (i j) -> (p i) j", i=L)
    for a, (r0, r1) in enumerate(((0, 32), (32, L))):
        eng = nc.sync if a == 0 else nc.scalar
        eng.dma_start(scores[r0:r1, :], sd_rows[r0:r1, :])
        nc.scalar.activation(expS[r0:r1, :], scores[r0:r1, :],
                             mybir.ActivationFunctionType.Exp,
                             accum_out=rowsum[r0:r1, :])
    recip = pool.tile([L, 1], dt, name=f"recip{u}", tag="recip")
    nc.vector.reciprocal(recip[:], rowsum[:])
    # A_r[i,j] = (exp * 1/rowsum) * mask_r, all 8 relations in one op
    attnW = pool.tile([L, NREL, L], dt, name=f"attnW{u}", tag="attnW")
    NH = NREL // 2
    for a in range(2):
        rsl = slice(a * NH, (a + 1) * NH)
        nc.vector.scalar_tensor_tensor(
            attnW[:, rsl, :],
            expS[:].rearrange("i (o j) -> i o j", o=1).broadcast_to([L, NH, L]),
            recip[:],
            maskt[:, rsl, :],
            op0=mybir.AluOpType.mult,
            op1=mybir.AluOpType.mult,
        )

    # ---- RGCN aggregation (h-slice); yall/W_root terms were banked above ----
    for r in range(NREL):
        nc.tensor.matmul(h_ps[:], yall[:, r * HS:(r + 1) * HS], attnW[:, r, :],
                         start=False, stop=(r == NREL - 1))
    hTs = pool.tile([HS, L], dt, name=f"hTs{u}", tag="hTs")
    nc.vector.tensor_scalar_add(hTs[:], h_ps[:], brg)

    # ---- AllGather h-slices -> full hT [256, 48] on every core ----
    ag_in = dram.tile([HS, L], dt, name=f"ag_in{u}", tag="ag_in")
    ag_out = dram.tile([H, L], dt, name=f"ag_out{u}", tag="ag_out")
    nc.sync.dma_start(ag_in[:], hTs[:])
    if collective:
        nc.gpsimd.collective_compute(
            "AllGather",
            mybir.AluOpType.bypass,
            replica_groups=[list(range(n_cores))],
            ins=[ag_in.opt()],
            outs=[ag_out.opt()],
        )
    else:
        # single-core stand-in for TimelineSim: replicate the slice 8x
        agw = ag_out[:].rearrange("(c p) f -> c p f", p=HS)
        for c in range(N_CORES):
            nc.sync.dma_start(agw[c], ag_in[:])
    hfull = pool.tile([128, 2, L], dt, name=f"hfull{u}", tag="hfull")
    agv = ag_out[:].rearrange("(t p) f -> t p f", p=128)
    nc.sync.dma_start(hfull[:, 0, :], agv[0])
    nc.scalar.dma_start(hfull[:, 1, :], agv[1])

    # ---- GraphConv (g-slice): out^T = W_self^T hT + (W_nbr^T s + b) ----
    sT = pool.tile([128, 2], dt, name=f"sT{u}", tag="sT")
    for t in range(2):
        nc.vector.reduce_sum(sT[:, t:t + 1], hfull[:, t, :],
                             axis=mybir.AxisListType.X)
    nb_ps = psum.tile([HS, 1], dt, name=f"nb_ps{u}", tag="mm_ps")
    for t in range(2):
        nc.tensor.matmul(nb_ps[:], wn(t).bitcast(dt), sT[:, t:t + 1],
                         start=(t == 0), stop=(t == 1))
    nbs = pool.tile([HS, 1], dt, name=f"nbs{u}", tag="nbs")
    nc.vector.tensor_scalar_add(nbs[:], nb_ps[:], bgc)

    out_ps = psum.tile([HS, L], dt, name=f"out_ps{u}", tag="mm_ps")
    for t in range(2):
        nc.tensor.matmul(out_ps[:], wsl(t).bitcast(dt), hfull[:, t, :],
                         start=(t == 0), stop=(t == 1))
    outs = pool.tile([HS, L], dt, name=f"outs{u}", tag="outs")
    nc.vector.tensor_scalar_add(outs[:], out_ps[:], nbs)
    nc.sync.dma_start(d["yout"].ap(), outs[:])


def build_program(n_cores=N_CORES, collective=True, repeat=1):
    """Build + schedule + compile the Bass program."""
    import concourse.bacc as bacc
    import concourse.mybir as mybir
    import concourse.tile as tile

    dt = mybir.dt.float32
    nc = bacc.Bacc("TRN2", debug=False, num_devices=n_cores)

    d = {}
    d["apack"] = nc.dram_tensor("apack", [2, 128, 304], mybir.dt.float32r,
                            kind="ExternalInput")
    d["rpack"] = nc.dram_tensor("rpack", [2, 128, 352], mybir.dt.float32r,
                            kind="ExternalInput")
    d["spack"] = nc.dram_tensor("spack", [128, 387], dt, kind="ExternalInput")
    d["yout"] = nc.dram_tensor("yout", [HS, L], dt, kind="ExternalOutput")

    with tile.TileContext(nc) as tc:
        with (
            tc.tile_pool(name="sbuf", bufs=1) as pool,
            tc.tile_pool(name="psum", bufs=1, space="PSUM") as psum,
            tc.tile_pool(name="dram", bufs=1, space="DRAM") as dram,
        ):
            for rep in range(repeat):
                _emit_body(nc, mybir, pool, psum, dram, d, rep, collective, n_cores)

    nc.compile()
    return nc


def _prepare_in_maps(global_features, speaker, Wq, Wk, v, W_rel, W_root, b_rgcn,
                     W_nbr, W_self, b_gcn):
    """Host-side routing: pick the <=8 live relation slices, build masks, pack
    per-core shards (h-slice of RGCN weights, g-slice of GraphConv weights)."""
    f32 = np.float32
    x = np.ascontiguousarray(global_features, dtype=f32)
    sp = np.asarray(speaker).astype(np.int64)
    n = L

    ii, jj = np.meshgrid(np.arange(n), np.arange(n), indexing="ij")
    direction = (ii >= jj).astype(np.int64)
    et = 2 * (sp[ii] * n + sp[jj]) + direction  # [48, 48] edge-type grid

    rel_ids = np.unique(et)
    assert len(rel_ids) <= NREL, f"{len(rel_ids)} live relations > {NREL}"
    masks = np.zeros((NREL, n, n), dtype=f32)
    rel_pad = np.full(NREL, rel_ids[0], dtype=np.int64)
    for s, rid in enumerate(rel_ids):
        masks[s] = (et == rid)
        rel_pad[s] = rid
    # padded slots keep zero masks -> contribute nothing

    W_used = np.ascontiguousarray(np.asarray(W_rel)[rel_pad], dtype=f32)  # [8,256,256]

    xt = np.ascontiguousarray(x.T).reshape(2, 128, L)
    wq = np.ascontiguousarray(Wq, dtype=f32).reshape(2, 128, A)
    wk = np.ascontiguousarray(Wk, dtype=f32).reshape(2, 128, A)
    maskw = np.ascontiguousarray(masks.transpose(1, 0, 2)).reshape(L, NREL * L)
    apack = np.ascontiguousarray(np.concatenate([xt, wq, wk], axis=2))
    W_root = np.asarray(W_root, dtype=f32)
    W_self = np.asarray(W_self, dtype=f32)
    W_nbr = np.asarray(W_nbr, dtype=f32)
    b_rgcn = np.asarray(b_rgcn, dtype=f32)
    b_gcn = np.asarray(b_gcn, dtype=f32)

    in_maps = []
    for c in range(N_CORES):
        sl = slice(c * HS, (c + 1) * HS)
        wrel_c = np.ascontiguousarray(
            W_used[:, :, sl].transpose(1, 0, 2)).reshape(2, 128, NREL * HS)
        rpack = np.ascontiguousarray(np.concatenate([
            wrel_c,
            W_root[:, sl].reshape(2, 128, HS),
            W_self[:, sl].reshape(2, 128, HS),
            W_nbr[:, sl].reshape(2, 128, HS),
        ], axis=2))
        spack = np.zeros((128, 3 + NREL * L), dtype=f32)
        spack[:, 0] = np.ascontiguousarray(v, dtype=f32).reshape(128)
        spack[0:L, 1:1 + NREL * L] = maskw
        spack[0:HS, 1 + NREL * L] = b_rgcn[sl]
        spack[0:HS, 2 + NREL * L] = b_gcn[sl]
        in_maps.append({"apack": apack, "rpack": rpack, "spack": spack})
    return in_maps


def kernel(global_features, speaker, Wq, Wk, v, W_rel, W_root, b_rgcn,
           W_nbr, W_self, b_gcn):
    global _compiled
    from concourse.bass_utils import run_bass_kernel_spmd

    if _compiled is None:
        _compiled = build_program()
    nc = _compiled

    in_maps = _prepare_in_maps(global_features, speaker, Wq, Wk, v, W_rel,
                               W_root, b_rgcn, W_nbr, W_self, b_gcn)
    res = run_bass_kernel_spmd(nc, in_maps, core_ids=list(range(N_CORES)))
    outT = np.concatenate([res.results[c]["yout"] for c in range(N_CORES)], axis=0)
    return np.ascontiguousarray(outT.T)



# revision 5
# speedup vs baseline: 4.1919x; 4.1919x over previous
"""DialogueGCN forward pass as a distributed Bass/Tile kernel on 8 TRN2 NeuronCores.

Math (reference): Bahdanau attention over utterance pairs -> per-edge softmax
weights; RGCN with per-relation weights W_rel[etype] + root term; GraphConv.

Key structural facts exploited:
  * etype = 2*(sp[i]*48 + sp[j]) + (i>=j) with speaker in {0,1} -> at most 8
    distinct relation types out of R=4608 are live. Only those 8 slices of the
    1.2GB W_rel are ever touched (host routes them to the devices).
  * The graph is fully connected, so the GraphConv neighbor sum is the same
    vector for every node: sum_i h_i.
  * agg^T = sum_r W_r^T x^T (attn*mask_r): 8 masked matmuls instead of a
    2304-edge gather/scatter.

Sharding (v2, collective-free): every core redundantly computes the attention
and the FULL RGCN hidden state h [48, 256] (weights shipped in bf16, ~1.4MB
per core, fully overlapped with compute), then computes only its 32-column
slice of the GraphConv output. No inter-core communication at all; the host
concatenates the 8 [32, 48] output slices and transposes.

The [1, 2304] -> [48, 48] scores reshape (cross-partition) is done with 48
tiny accumulating one-hot outer-product matmuls on the PE instead of a DRAM
round-trip.
"""
import numpy as np

L = 48
D = 256
H = 256
G = 256
A = 128
N_CORES = 8
GS = G // N_CORES  # 32 output columns per core
NREL = 8

_compiled = None


def _emit_body(nc, mybir, pool, psum, dram, d, rep, n_cores):
    """Emit one full forward pass. `d` maps dram-tensor names to handles."""
    bf = mybir.dt.bfloat16
    f32 = mybir.dt.float32
    u = f"_{rep}"

    # ---- input DMAs ----
    # hwDGE stream (SP + ACT queues): attention pack first, then 2 W_rel groups
    # swDGE stream (Pool queue): graphconv/root weights + remaining W_rel groups
    apk = pool.tile([128, 2, 305], bf, name=f"apk{u}", tag="apk")
    nc.sync.dma_start(apk[:], d["apack"].ap())
    wpk = []
    for g in range(4):
        w = pool.tile([128, 2, 512], bf, name=f"wpk{g}{u}", tag=f"wpk{g}")
        wpk.append(w)
    nc.sync.dma_start(wpk[0][:], d["wpk0"].ap())
    nc.sync.dma_start(wpk[1][:], d["wpk1"].ap())
    spk = pool.tile([L, NREL * L], bf, name=f"spk{u}", tag="spk")
    nc.scalar.dma_start(spk[:], d["spack"].ap())
    idr = pool.tile([1, L * L], bf, name=f"idr{u}", tag="idr")
    nc.scalar.dma_start(idr[:], d["idrows"].ap())
    bpk = pool.tile([128, 3], f32, name=f"bpk{u}", tag="bpk")
    nc.scalar.dma_start(bpk[:], d["bpk"].ap())
    wts = pool.tile([128, 2, 320], bf, name=f"wts{u}", tag="wts")
    nc.gpsimd.dma_start(wts[:], d["wtpack"].ap())
    nc.gpsimd.dma_start(wpk[2][:], d["wpk2"].ap())
    nc.gpsimd.dma_start(wpk[3][:], d["wpk3"].ap())

    # ---- Bahdanau attention: scores[i,j] = v . tanh(qT[:,i]+kT[:,j]) ----
    qT_ps = psum.tile([128, L], f32, name=f"qT{u}", tag="psA")
    for t in range(2):
        nc.tensor.matmul(qT_ps[:], apk[:, t, 48:176], apk[:, t, 0:48],
                         start=(t == 0), stop=(t == 1))
    kT_ps = psum.tile([128, L], f32, name=f"kT{u}", tag="psB")
    for t in range(2):
        nc.tensor.matmul(kT_ps[:], apk[:, t, 176:304], apk[:, t, 0:48],
                         start=(t == 0), stop=(t == 1))
    qTs = pool.tile([128, L, 1], bf, name=f"qTs{u}", tag="qTs")
    nc.vector.tensor_copy(qTs[:, :, 0], qT_ps[:])
    kTs = pool.tile([128, 1, L], bf, name=f"kTs{u}", tag="kTs")
    nc.vector.tensor_copy(kTs[:, 0, :], kT_ps[:])

    # broadcast add (DVE) + tanh (ACT), 4 chunks of 12 query rows
    NCH = 4
    CL = L // NCH
    bigT = pool.tile([128, L, L], bf, name=f"bigT{u}", tag="bigT")
    tanhT = pool.tile([128, L * L], bf, name=f"tanhT{u}", tag="tanhT")
    for h in range(NCH):
        isl = slice(h * CL, (h + 1) * CL)
        nc.vector.tensor_tensor(
            bigT[:, isl, :],
            qTs[:, isl, :].broadcast_to([128, CL, L]),
            kTs[:].broadcast_to([128, CL, L]),
            op=mybir.AluOpType.add,
        )
        nc.scalar.activation(tanhT[:, h * CL * L:(h + 1) * CL * L],
                             bigT[:, isl, :].rearrange("p i j -> p (i j)"),
                             mybir.ActivationFunctionType.Tanh)

    # ---- yall[i, r*256+h] = (x @ W_r)[i, h]: pipelined behind W_rel DMAs ----
    yall = pool.tile([L, NREL * H], bf, name=f"yall{u}", tag="yall")
    yps = []
    for g in range(4):
        yp = psum.tile([L, 512], f32, name=f"yp{g}{u}", tag=f"yp{g % 2}")
        for t in range(2):
            nc.tensor.matmul(yp[:], apk[:, t, 0:48], wpk[g][:, t, :],
                             start=(t == 0), stop=(t == 1))
        yps.append(yp)

    # ---- score matmuls: v^T @ tanh in 5 chunks of <=480 (bank-safe) ----
    vvr = apk[:, 0, 304:305]
    NSC = 5
    sc_ps = []
    for k in range(NSC):
        lo, hi = 480 * k, min(480 * (k + 1), L * L)
        sc = psum.tile([1, hi - lo], f32, name=f"sc{k}{u}", tag=f"sc{k % 2}")
        nc.tensor.matmul(sc[:], vvr, tanhT[:, lo:hi], start=True, stop=True)
        sc_ps.append(sc)

    # PSUM -> SBUF drains. DVE handles score chunks after the adds; ACT takes
    # two after the tanh chain; yall drains split DVE/ACT as they free up.
    scoresb = pool.tile([1, L * L], bf, name=f"scb{u}", tag="scb")

    def psum_drain(eng, dst, src):
        if eng is nc.scalar:
            nc.scalar.copy(dst, src)
        else:
            eng.tensor_copy(dst, src)

    copy_engs = [nc.vector, nc.scalar, nc.vector, nc.scalar, nc.vector]
    for k in range(NSC):
        lo, hi = 480 * k, min(480 * (k + 1), L * L)
        psum_drain(copy_engs[k], scoresb[:, lo:hi], sc_ps[k][:])
    drain_engs = [nc.vector, nc.scalar, nc.vector, nc.scalar]
    for g in range(4):
        psum_drain(drain_engs[g], yall[:, 512 * g:512 * (g + 1)], yps[g][:])

    # ---- reshape [1, 2304] -> [48, 48] via 48 one-hot outer products ----
    sq_ps = psum.tile([L, L], f32, name=f"sq{u}", tag="sq_ps")
    for i in range(L):
        nc.tensor.matmul(sq_ps[:], idr[:, L * i:L * (i + 1)],
                         scoresb[:, L * i:L * (i + 1)],
                         start=(i == 0), stop=(i == L - 1))

    # ---- softmax + masked per-relation attention weights ----
    expS = pool.tile([L, 1, L], bf, name=f"expS{u}", tag="expS")
    rowsum = pool.tile([L, 1], f32, name=f"rs{u}", tag="rowsum")
    nc.scalar.activation(expS[:, 0, :], sq_ps[:],
                         mybir.ActivationFunctionType.Exp,
                         accum_out=rowsum[:])
    recip = pool.tile([L, 1], f32, name=f"rc{u}", tag="recip")
    nc.vector.reciprocal(recip[:], rowsum[:])
    maskt = spk[:].rearrange("i (r j) -> i r j", r=NREL)
    attnW = pool.tile([L, NREL, L], bf, name=f"attnW{u}", tag="attnW")
    NH = NREL // 2
    for a in range(2):
        rsl = slice(a * NH, (a + 1) * NH)
        nc.vector.scalar_tensor_tensor(
            attnW[:, rsl, :],
            expS[:].broadcast_to([L, NH, L]),
            recip[:],
            maskt[:, rsl, :],
            op0=mybir.AluOpType.mult,
            op1=mybir.AluOpType.mult,
        )

    # ---- h^T = W_root^T x^T + sum_r yall_r^T A_r + b_rgcn, in 2 H-halves ----
    hT_ps = [psum.tile([128, L], f32, name=f"hT{t}{u}", tag=("psA", "psB")[t])
             for t in range(2)]
    for t in range(2):
        for tt in range(2):
            nc.tensor.matmul(hT_ps[t][:], wts[:, tt, 128 * t:128 * (t + 1)],
                             apk[:, tt, 0:48], start=(tt == 0), stop=False)
    for t in range(2):
        for r in range(NREL):
            nc.tensor.matmul(hT_ps[t][:],
                             yall[:, H * r + 128 * t:H * r + 128 * (t + 1)],
                             attnW[:, r, :], start=False, stop=(r == NREL - 1))
    hT = pool.tile([128, 2, L], bf, name=f"hTs{u}", tag="hTs")
    for t in range(2):
        nc.vector.tensor_scalar_add(hT[:, t, :], hT_ps[t][:], bpk[:, t:t + 1])

    # ---- GraphConv (g-slice): out^T = W_self_c^T hT + (W_nbr_c^T s + b) ----
    sT = pool.tile([128, 2], bf, name=f"sT{u}", tag="sT")
    with nc.allow_low_precision(reason="48-term sum in bf16, |h|~1"):
        for t in range(2):
            nc.vector.reduce_sum(sT[:, t:t + 1], hT[:, t, :],
                                 axis=mybir.AxisListType.X)
    nb_ps = psum.tile([GS, 1], f32, name=f"nb{u}", tag="yp0")
    for t in range(2):
        nc.tensor.matmul(nb_ps[:], wts[:, t, 288:320], sT[:, t:t + 1],
                         start=(t == 0), stop=(t == 1))
    nbs = pool.tile([GS, 1], f32, name=f"nbs{u}", tag="nbs")
    nc.vector.tensor_scalar_add(nbs[:], nb_ps[:], bpk[0:GS, 2:3])
    out_ps = psum.tile([GS, L], f32, name=f"op{u}", tag="yp1")
    for t in range(2):
        nc.tensor.matmul(out_ps[:], wts[:, t, 256:288], hT[:, t, :],
                         start=(t == 0), stop=(t == 1))
    outs = pool.tile([GS, L], f32, name=f"outs{u}", tag="outs")
    nc.vector.tensor_scalar_add(outs[:], out_ps[:], nbs)
    nc.sync.dma_start(d["yout"].ap(), outs[:])


def build_program(n_cores=N_CORES, collective=False, repeat=1):
    """Build + schedule + compile the Bass program."""
    import concourse.bacc as bacc
    import concourse.mybir as mybir
    import concourse.tile as tile

    bf = mybir.dt.bfloat16
    f32 = mybir.dt.float32
    nc = bacc.Bacc("TRN2", debug=False, num_devices=n_cores)

    d = {}
    d["apack"] = nc.dram_tensor("apack", [128, 2, 305], bf, kind="ExternalInput")
    for g in range(4):
        d[f"wpk{g}"] = nc.dram_tensor(f"wpk{g}", [128, 2, 512], bf,
                                      kind="ExternalInput")
    d["wtpack"] = nc.dram_tensor("wtpack", [128, 2, 320], bf,
                                 kind="ExternalInput")
    d["spack"] = nc.dram_tensor("spack", [L, NREL * L], bf,
                                kind="ExternalInput")
    d["idrows"] = nc.dram_tensor("idrows", [1, L * L], bf,
                                 kind="ExternalInput")
    d["bpk"] = nc.dram_tensor("bpk", [128, 3], f32, kind="ExternalInput")
    d["yout"] = nc.dram_tensor("yout", [GS, L], f32, kind="ExternalOutput")

    with tile.TileContext(nc) as tc:
        with (
            tc.tile_pool(name="sbuf", bufs=1) as pool,
            tc.tile_pool(name="psum", bufs=1, space="PSUM") as psum,
            tc.tile_pool(name="dram", bufs=1, space="DRAM") as dram,
        ):
            for rep in range(repeat):
                _emit_body(nc, mybir, pool, psum, dram, d, rep, n_cores)

    nc.compile()
    return nc


def _prepare_in_maps(global_features, speaker, Wq, Wk, v, W_rel, W_root, b_rgcn,
                     W_nbr, W_self, b_gcn):
    """Host-side routing: pick the <=8 live relation slices, build masks, pack
    per-core shards (bf16 weights; each core gets the full RGCN weights and
    its 32-column slice of the GraphConv weights)."""
    import concourse.mybir as mybir
    bf = mybir.dt.np(mybir.dt.bfloat16)
    f32 = np.float32
    x = np.ascontiguousarray(global_features, dtype=f32)
    sp = np.asarray(speaker).astype(np.int64)
    n = L

    ii, jj = np.meshgrid(np.arange(n), np.arange(n), indexing="ij")
    direction = (ii >= jj).astype(np.int64)
    et = 2 * (sp[ii] * n + sp[jj]) + direction  # [48, 48] edge-type grid

    rel_ids = np.unique(et)
    assert len(rel_ids) <= NREL, f"{len(rel_ids)} live relations > {NREL}"
    masks = np.zeros((NREL, n, n), dtype=f32)
    rel_pad = np.full(NREL, rel_ids[0], dtype=np.int64)
    for s, rid in enumerate(rel_ids):
        masks[s] = (et == rid)
        rel_pad[s] = rid
    # padded slots keep zero masks -> contribute nothing

    W_used = np.asarray(W_rel)[rel_pad].astype(f32)       # [8, 256, 256]
    Wq = np.asarray(Wq, dtype=f32)
    Wk = np.asarray(Wk, dtype=f32)
    W_root = np.asarray(W_root, dtype=f32)
    W_self = np.asarray(W_self, dtype=f32)
    W_nbr = np.asarray(W_nbr, dtype=f32)
    b_rgcn = np.asarray(b_rgcn, dtype=f32)
    b_gcn = np.asarray(b_gcn, dtype=f32)
    vv = np.asarray(v, dtype=f32)

    # attention pack [128, 2, 305]: x^T halves, Wq, Wk, v
    apack = np.zeros((128, 2, 305), dtype=f32)
    xt = x.T.reshape(2, 128, n)                            # [t, p, i]
    for t in range(2):
        apack[:, t, 0:48] = xt[t]
        apack[:, t, 48:176] = Wq[128 * t:128 * (t + 1), :]
        apack[:, t, 176:304] = Wk[128 * t:128 * (t + 1), :]
    apack[:, 0, 304] = vv
    apack = apack.astype(bf)

    # W_rel rhs pack: wrel[d, r*256+h]; 4 groups of 512 cols, [p, t, 512]
    wrel = W_used.transpose(1, 0, 2).reshape(D, NREL * H)  # [d, r*H+h]
    wpks = []
    for g in range(4):
        cols = wrel[:, 512 * g:512 * (g + 1)].reshape(2, 128, 512)
        wpks.append(np.ascontiguousarray(cols.transpose(1, 0, 2)).astype(bf))

    maskw = np.ascontiguousarray(
        masks.transpose(1, 0, 2)).reshape(n, NREL * n).astype(bf)
    idrows = np.eye(n, dtype=f32).reshape(1, n * n).astype(bf)

    in_maps = []
    for c in range(N_CORES):
        sl = slice(c * GS, (c + 1) * GS)
        wtpack = np.zeros((128, 2, 320), dtype=f32)
        for t in range(2):
            wtpack[:, t, 0:256] = W_root[128 * t:128 * (t + 1), :]
            wtpack[:, t, 256:288] = W_self[128 * t:128 * (t + 1), sl]
            wtpack[:, t, 288:320] = W_nbr[128 * t:128 * (t + 1), sl]
        bpk = np.zeros((128, 3), dtype=f32)
        bpk[:, 0] = b_rgcn[0:128]
        bpk[:, 1] = b_rgcn[128:256]
        bpk[0:GS, 2] = b_gcn[sl]
        in_maps.append({
            "apack": apack,
            "wpk0": wpks[0], "wpk1": wpks[1],
            "wpk2": wpks[2], "wpk3": wpks[3],
            "wtpack": wtpack.astype(bf),
            "spack": maskw,
            "idrows": idrows,
            "bpk": bpk,
        })
    return in_maps


def kernel(global_features, speaker, Wq, Wk, v, W_rel, W_root, b_rgcn,
           W_nbr, W_self, b_gcn):
    global _compiled
    from concourse.bass_utils import run_bass_kernel_spmd

    if _compiled is None:
        _compiled = build_program()
    nc = _compiled

    in_maps = _prepare_in_maps(global_features, speaker, Wq, Wk, v, W_rel,
                               W_root, b_rgcn, W_nbr, W_self, b_gcn)
    res = run_bass_kernel_spmd(nc, in_maps, core_ids=list(range(N_CORES)))
    outT = np.concatenate([res.results[c]["yout"] for c in range(N_CORES)], axis=0)
    return np.ascontiguousarray(outT.T)


# revision 10
# speedup vs baseline: 5.0348x; 1.2011x over previous
"""DialogueGCN forward pass as a distributed Bass/Tile kernel on 8 TRN2 NeuronCores.

Math (reference): Bahdanau attention over utterance pairs -> per-edge softmax
weights; RGCN with per-relation weights W_rel[etype] + root term; GraphConv.

Key structural facts exploited:
  * etype = 2*(sp[i]*48 + sp[j]) + (i>=j) with speaker in {0,1} -> at most 8
    distinct relation types out of R=4608 are live. Only those 8 slices of the
    1.2GB W_rel are ever touched (host routes them to the devices).
  * The graph is fully connected, so the GraphConv neighbor sum is the same
    vector for every node: sum_i h_i.
  * agg^T = sum_r W_r^T x^T (attn*mask_r): 8 masked matmuls instead of a
    2304-edge gather/scatter.

Sharding (v2, collective-free): every core redundantly computes the attention
and the FULL RGCN hidden state h [48, 256] (weights shipped in bf16, ~1.4MB
per core, fully overlapped with compute), then computes only its 32-column
slice of the GraphConv output. No inter-core communication at all; the host
concatenates the 8 [32, 48] output slices and transposes.

The [1, 2304] -> [48, 48] scores reshape (cross-partition) is done with 48
tiny accumulating one-hot outer-product matmuls on the PE instead of a DRAM
round-trip.
"""
import numpy as np

L = 48
D = 256
H = 256
G = 256
A = 128
N_CORES = 8
GS = G // N_CORES  # 32 output columns per core
NREL = 8

_compiled = None


def _emit_body(nc, mybir, pool, psum, dram, d, rep, n_cores):
    """Emit one full forward pass. `d` maps dram-tensor names to handles."""
    bf = mybir.dt.bfloat16
    f32 = mybir.dt.float32
    u = f"_{rep}"

    # ---- input DMAs ----
    # hwDGE stream (SP + ACT queues): attention pack first, then 2 W_rel groups
    # swDGE stream (Pool queue): graphconv/root weights + remaining W_rel groups
    apk = pool.tile([128, 2, 305], bf, name=f"apk{u}", tag="apk")
    nc.sync.dma_start(apk[:], d["apack"].ap())
    wpk = []
    for g in range(4):
        w = pool.tile([128, 2, 512], bf, name=f"wpk{g}{u}", tag=f"wpk{g}")
        wpk.append(w)
    nc.sync.dma_start(wpk[0][:], d["wpk0"].ap())
    nc.sync.dma_start(wpk[1][:], d["wpk1"].ap())
    spk = pool.tile([L, NREL * L], bf, name=f"spk{u}", tag="spk")
    nc.scalar.dma_start(spk[:], d["spack"].ap())
    bpk = pool.tile([128, 3], f32, name=f"bpk{u}", tag="bpk")
    nc.scalar.dma_start(bpk[:], d["bpk"].ap())
    wts = pool.tile([128, 2, 320], bf, name=f"wts{u}", tag="wts")
    nc.gpsimd.dma_start(wts[:], d["wtpack"].ap())
    nc.gpsimd.dma_start(wpk[2][:], d["wpk2"].ap())
    nc.gpsimd.dma_start(wpk[3][:], d["wpk3"].ap())

    # ---- Bahdanau attention: scores[i,j] = v . tanh(qT[:,i]+kT[:,j]) ----
    qT_ps = psum.tile([128, L], f32, name=f"qT{u}", tag="psA")
    for t in range(2):
        nc.tensor.matmul(qT_ps[:], apk[:, t, 48:176], apk[:, t, 0:48],
                         start=(t == 0), stop=(t == 1))
    kT_ps = psum.tile([128, L], f32, name=f"kT{u}", tag="psB")
    for t in range(2):
        nc.tensor.matmul(kT_ps[:], apk[:, t, 176:304], apk[:, t, 0:48],
                         start=(t == 0), stop=(t == 1))
    qTs = pool.tile([128, L, 1], bf, name=f"qTs{u}", tag="qTs")
    nc.scalar.copy(qTs[:, :, 0], qT_ps[:])
    kTs = pool.tile([128, 1, L], bf, name=f"kTs{u}", tag="kTs")
    nc.scalar.copy(kTs[:, 0, :], kT_ps[:])

    # broadcast add (DVE) + tanh (ACT), 4 chunks of 12 query rows
    NCH = 4
    CL = L // NCH
    bigT = pool.tile([128, L, L], bf, name=f"bigT{u}", tag="bigT")
    tanhT = pool.tile([128, L, L], bf, name=f"tanhT{u}", tag="tanhT")
    for h in range(NCH):
        isl = slice(h * CL, (h + 1) * CL)
        nc.vector.tensor_tensor(
            bigT[:, isl, :],
            qTs[:, isl, :].broadcast_to([128, CL, L]),
            kTs[:].broadcast_to([128, CL, L]),
            op=mybir.AluOpType.add,
        )
        nc.scalar.activation(
            tanhT[:, isl, :].rearrange("p i j -> p (i j)"),
            bigT[:, isl, :].rearrange("p i j -> p (i j)"),
            mybir.ActivationFunctionType.Tanh)

    # ---- yall[i, r*256+h] = (x @ W_r)[i, h]: pipelined behind W_rel DMAs ----
    yall = pool.tile([L, NREL * H], bf, name=f"yall{u}", tag="yall")
    yps = []
    for g in range(4):
        yp = psum.tile([L, 512], f32, name=f"yp{g}{u}", tag=f"yp{g % 2}")
        for t in range(2):
            nc.tensor.matmul(yp[:], apk[:, t, 0:48], wpk[g][:, t, :],
                             start=(t == 0), stop=(t == 1))
        yps.append(yp)

    # ---- yall PSUM -> SBUF drains (split ACT/DVE) ----
    def psum_drain(eng, dst, src):
        if eng is nc.scalar:
            nc.scalar.copy(dst, src)
        else:
            eng.tensor_copy(dst, src)

    drain_engs = [nc.scalar, nc.scalar, nc.vector, nc.scalar]
    for g in range(4):
        psum_drain(drain_engs[g], yall[:, 512 * g:512 * (g + 1)], yps[g][:])

    # ---- scores[i, j] = v . tanhT[:, i, j]: one tiny N=1 matmul per column
    # (strided lhsT); writes the [48, 48] score grid straight into PSUM with
    # no cross-partition reshape needed.
    vvr = apk[:, 0, 304:305]
    sq_ps = psum.tile([L, L], f32, name=f"sq{u}", tag="sq_ps")
    for j in range(L):
        nc.tensor.matmul(sq_ps[:, j:j + 1], tanhT[:, :, j], vvr,
                         start=True, stop=True)

    # ---- softmax + masked per-relation attention weights ----
    expS = pool.tile([L, 1, L], bf, name=f"expS{u}", tag="expS")
    rowsum = pool.tile([L, 1], f32, name=f"rs{u}", tag="rowsum")
    nc.scalar.activation(expS[:, 0, :], sq_ps[:],
                         mybir.ActivationFunctionType.Exp,
                         accum_out=rowsum[:])
    recip = pool.tile([L, 1], f32, name=f"rc{u}", tag="recip")
    nc.vector.reciprocal(recip[:], rowsum[:])
    maskt = spk[:].rearrange("i (r j) -> i r j", r=NREL)
    attnW = pool.tile([L, NREL, L], bf, name=f"attnW{u}", tag="attnW")
    NH = NREL // 2
    for a in range(2):
        rsl = slice(a * NH, (a + 1) * NH)
        nc.vector.scalar_tensor_tensor(
            attnW[:, rsl, :],
            expS[:].broadcast_to([L, NH, L]),
            recip[:],
            maskt[:, rsl, :],
            op0=mybir.AluOpType.mult,
            op1=mybir.AluOpType.mult,
        )

    # ---- h^T = W_root^T x^T + sum_r yall_r^T A_r + b_rgcn, in 2 H-halves ----
    hT_ps = [psum.tile([128, L], f32, name=f"hT{t}{u}", tag=("psA", "psB")[t])
             for t in range(2)]
    for t in range(2):
        for tt in range(2):
            nc.tensor.matmul(hT_ps[t][:], wts[:, tt, 128 * t:128 * (t + 1)],
                             apk[:, tt, 0:48], start=(tt == 0), stop=False)
    for t in range(2):
        for r in range(NREL):
            nc.tensor.matmul(hT_ps[t][:],
                             yall[:, H * r + 128 * t:H * r + 128 * (t + 1)],
                             attnW[:, r, :], start=False, stop=(r == NREL - 1))
    hT = pool.tile([128, 2, L], bf, name=f"hTs{u}", tag="hTs")
    for t in range(2):
        nc.vector.tensor_scalar_add(hT[:, t, :], hT_ps[t][:], bpk[:, t:t + 1])

    # ---- GraphConv (g-slice): out^T = W_self_c^T hT + (W_nbr_c^T s + b) ----
    sT = pool.tile([128, 2], bf, name=f"sT{u}", tag="sT")
    with nc.allow_low_precision(reason="48-term sum in bf16, |h|~1"):
        for t in range(2):
            nc.vector.reduce_sum(sT[:, t:t + 1], hT[:, t, :],
                                 axis=mybir.AxisListType.X)
    nb_ps = psum.tile([GS, 1], f32, name=f"nb{u}", tag="yp0")
    for t in range(2):
        nc.tensor.matmul(nb_ps[:], wts[:, t, 288:320], sT[:, t:t + 1],
                         start=(t == 0), stop=(t == 1))
    nbs = pool.tile([GS, 1], f32, name=f"nbs{u}", tag="nbs")
    nc.vector.tensor_scalar_add(nbs[:], nb_ps[:], bpk[0:GS, 2:3])
    out_ps = psum.tile([GS, L], f32, name=f"op{u}", tag="yp1")
    for t in range(2):
        nc.tensor.matmul(out_ps[:], wts[:, t, 256:288], hT[:, t, :],
                         start=(t == 0), stop=(t == 1))
    outs = pool.tile([GS, L], f32, name=f"outs{u}", tag="outs")
    nc.vector.tensor_scalar_add(outs[:], out_ps[:], nbs)
    nc.sync.dma_start(d["yout"].ap(), outs[:])


def build_program(n_cores=N_CORES, collective=False, repeat=1):
    """Build + schedule + compile the Bass program."""
    import concourse.bacc as bacc
    import concourse.mybir as mybir
    import concourse.tile as tile

    bf = mybir.dt.bfloat16
    f32 = mybir.dt.float32
    nc = bacc.Bacc("TRN2", debug=False, num_devices=n_cores)

    d = {}
    d["apack"] = nc.dram_tensor("apack", [128, 2, 305], bf, kind="ExternalInput")
    for g in range(4):
        d[f"wpk{g}"] = nc.dram_tensor(f"wpk{g}", [128, 2, 512], bf,
                                      kind="ExternalInput")
    d["wtpack"] = nc.dram_tensor("wtpack", [128, 2, 320], bf,
                                 kind="ExternalInput")
    d["spack"] = nc.dram_tensor("spack", [L, NREL * L], bf,
                                kind="ExternalInput")
    d["bpk"] = nc.dram_tensor("bpk", [128, 3], f32, kind="ExternalInput")
    d["yout"] = nc.dram_tensor("yout", [GS, L], f32, kind="ExternalOutput")

    with tile.TileContext(nc) as tc:
        with (
            tc.tile_pool(name="sbuf", bufs=1) as pool,
            tc.tile_pool(name="psum", bufs=1, space="PSUM") as psum,
            tc.tile_pool(name="dram", bufs=1, space="DRAM") as dram,
        ):
            for rep in range(repeat):
                _emit_body(nc, mybir, pool, psum, dram, d, rep, n_cores)

    nc.compile()
    return nc


def _prepare_in_maps(global_features, speaker, Wq, Wk, v, W_rel, W_root, b_rgcn,
                     W_nbr, W_self, b_gcn):
    """Host-side routing: pick the <=8 live relation slices, build masks, pack
    per-core shards (bf16 weights; each core gets the full RGCN weights and
    its 32-column slice of the GraphConv weights)."""
    import concourse.mybir as mybir
    bf = mybir.dt.np(mybir.dt.bfloat16)
    f32 = np.float32
    x = np.ascontiguousarray(global_features, dtype=f32)
    sp = np.asarray(speaker).astype(np.int64)
    n = L

    ii, jj = np.meshgrid(np.arange(n), np.arange(n), indexing="ij")
    direction = (ii >= jj).astype(np.int64)
    et = 2 * (sp[ii] * n + sp[jj]) + direction  # [48, 48] edge-type grid

    rel_ids = np.unique(et)
    assert len(rel_ids) <= NREL, f"{len(rel_ids)} live relations > {NREL}"
    masks = np.zeros((NREL, n, n), dtype=f32)
    rel_pad = np.full(NREL, rel_ids[0], dtype=np.int64)
    for s, rid in enumerate(rel_ids):
        masks[s] = (et == rid)
        rel_pad[s] = rid
    # padded slots keep zero masks -> contribute nothing

    W_used = np.asarray(W_rel)[rel_pad].astype(f32)       # [8, 256, 256]
    Wq = np.asarray(Wq, dtype=f32)
    Wk = np.asarray(Wk, dtype=f32)
    W_root = np.asarray(W_root, dtype=f32)
    W_self = np.asarray(W_self, dtype=f32)
    W_nbr = np.asarray(W_nbr, dtype=f32)
    b_rgcn = np.asarray(b_rgcn, dtype=f32)
    b_gcn = np.asarray(b_gcn, dtype=f32)
    vv = np.asarray(v, dtype=f32)

    # attention pack [128, 2, 305]: x^T halves, Wq, Wk, v
    apack = np.zeros((128, 2, 305), dtype=f32)
    xt = x.T.reshape(2, 128, n)                            # [t, p, i]
    for t in range(2):
        apack[:, t, 0:48] = xt[t]
        apack[:, t, 48:176] = Wq[128 * t:128 * (t + 1), :]
        apack[:, t, 176:304] = Wk[128 * t:128 * (t + 1), :]
    apack[:, 0, 304] = vv
    apack = apack.astype(bf)

    # W_rel rhs pack: wrel[d, r*256+h]; 4 groups of 512 cols, [p, t, 512]
    wrel = W_used.transpose(1, 0, 2).reshape(D, NREL * H)  # [d, r*H+h]
    wpks = []
    for g in range(4):
        cols = wrel[:, 512 * g:512 * (g + 1)].reshape(2, 128, 512)
        wpks.append(np.ascontiguousarray(cols.transpose(1, 0, 2)).astype(bf))

    maskw = np.ascontiguousarray(
        masks.transpose(1, 0, 2)).reshape(n, NREL * n).astype(bf)

    in_maps = []
    for c in range(N_CORES):
        sl = slice(c * GS, (c + 1) * GS)
        wtpack = np.zeros((128, 2, 320), dtype=f32)
        for t in range(2):
            wtpack[:, t, 0:256] = W_root[128 * t:128 * (t + 1), :]
            wtpack[:, t, 256:288] = W_self[128 * t:128 * (t + 1), sl]
            wtpack[:, t, 288:320] = W_nbr[128 * t:128 * (t + 1), sl]
        bpk = np.zeros((128, 3), dtype=f32)
        bpk[:, 0] = b_rgcn[0:128]
        bpk[:, 1] = b_rgcn[128:256]
        bpk[0:GS, 2] = b_gcn[sl]
        in_maps.append({
            "apack": apack,
            "wpk0": wpks[0], "wpk1": wpks[1],
            "wpk2": wpks[2], "wpk3": wpks[3],
            "wtpack": wtpack.astype(bf),
            "spack": maskw,
            "bpk": bpk,
        })
    return in_maps


def kernel(global_features, speaker, Wq, Wk, v, W_rel, W_root, b_rgcn,
           W_nbr, W_self, b_gcn):
    global _compiled
    from concourse.bass_utils import run_bass_kernel_spmd

    if _compiled is None:
        _compiled = build_program()
    nc = _compiled

    in_maps = _prepare_in_maps(global_features, speaker, Wq, Wk, v, W_rel,
                               W_root, b_rgcn, W_nbr, W_self, b_gcn)
    res = run_bass_kernel_spmd(nc, in_maps, core_ids=list(range(N_CORES)))
    outT = np.concatenate([res.results[c]["yout"] for c in range(N_CORES)], axis=0)
    return np.ascontiguousarray(outT.T)


# revision 20
# speedup vs baseline: 29.6628x; 5.8915x over previous
"""DialogueGCN forward pass as a distributed Bass/Tile kernel on 8 TRN2 NeuronCores.

Math (reference): Bahdanau attention over utterance pairs -> per-edge softmax
weights; RGCN with per-relation weights W_rel[etype] + root term; GraphConv.

Key structural facts exploited:
  * etype = 2*(sp[i]*48 + sp[j]) + (i>=j) with speaker in {0,1} -> at most 8
    distinct relation types out of R=4608 are live. Only those 8 slices of the
    1.2GB W_rel are ever touched (host routes them to the devices).
  * The graph is fully connected, so the GraphConv neighbor sum is the same
    vector for every node: sum_i h_i.
  * agg^T = sum_r W_r^T x^T (attn*mask_r): 8 masked matmuls instead of a
    2304-edge gather/scatter.

Sharding (v2, collective-free): every core redundantly computes the attention
and the FULL RGCN hidden state h [48, 256] (weights shipped in bf16, ~1.4MB
per core, fully overlapped with compute), then computes only its 32-column
slice of the GraphConv output. No inter-core communication at all; the host
concatenates the 8 [32, 48] output slices and transposes.

The [1, 2304] -> [48, 48] scores reshape (cross-partition) is done with 48
tiny accumulating one-hot outer-product matmuls on the PE instead of a DRAM
round-trip.
"""
import numpy as np

L = 48
D = 256
H = 256
G = 256
A = 128
N_CORES = 8
GS = G // N_CORES  # 32 output columns per core
NREL = 8

_compiled = None


def _emit_body(nc, mybir, tc, pool, psum, dram, d, rep, n_cores):
    """Emit one full forward pass. `d` maps dram-tensor names to handles."""
    bf = mybir.dt.bfloat16
    f32 = mybir.dt.float32
    u = f"_{rep}"

    # ---- input DMAs ----
    # hwDGE stream (SP + ACT queues): attention pack first, then 2 W_rel groups
    # swDGE stream (Pool queue): graphconv/root weights + remaining W_rel groups
    apk = pool.tile([128, 2, 305], bf, name=f"apk{u}", tag="apk")
    nc.sync.dma_start(apk[:], d["apack"].ap())
    wpk = []
    for g in range(4):
        w = pool.tile([128, 2, 512], bf, name=f"wpk{g}{u}", tag=f"wpk{g}")
        wpk.append(w)
    nc.sync.dma_start(wpk[0][:], d["wpk0"].ap())
    nc.sync.dma_start(wpk[1][:], d["wpk1"].ap())
    spk = pool.tile([L, NREL * L], bf, name=f"spk{u}", tag="spk")
    nc.sync.dma_start(spk[:], d["spack"].ap())
    bpk = pool.tile([128, 3], f32, name=f"bpk{u}", tag="bpk")
    nc.sync.dma_start(bpk[:], d["bpk"].ap())
    wts = pool.tile([128, 2, 320], bf, name=f"wts{u}", tag="wts")
    nc.gpsimd.dma_start(wts[:], d["wtpack"].ap())
    nc.gpsimd.dma_start(wpk[2][:], d["wpk2"].ap())
    nc.gpsimd.dma_start(wpk[3][:], d["wpk3"].ap())

    # Wait hints below are compile-time scheduling keys only (they order the
    # per-engine instruction streams); runtime is still semaphore-driven.
    W = tc.tile_wait_until

    # ---- Bahdanau attention: scores[i,j] = v . tanh(qT[:,i]+kT[:,j]) ----
    qT_ps = psum.tile([128, L], f32, name=f"qT{u}", tag="psA")
    kT_ps = psum.tile([128, L], f32, name=f"kT{u}", tag="psB")
    with W(0.01):
        for t in range(2):
            nc.tensor.matmul(qT_ps[:], apk[:, t, 48:176], apk[:, t, 0:48],
                             start=(t == 0), stop=(t == 1))
        for t in range(2):
            nc.tensor.matmul(kT_ps[:], apk[:, t, 176:304], apk[:, t, 0:48],
                             start=(t == 0), stop=(t == 1))
    qTs = pool.tile([128, L, 1], bf, name=f"qTs{u}", tag="qTs")
    kTs = pool.tile([128, 1, L], bf, name=f"kTs{u}", tag="kTs")
    with W(0.015):
        nc.scalar.copy(qTs[:, :, 0], qT_ps[:])
        nc.vector.tensor_copy(kTs[:, 0, :], kT_ps[:])

    # broadcast add (DVE x3 + gpsimd x1) + tanh (ACT), 4 chunks of 12 rows
    NCH = 4
    CL = L // NCH
    bigT = pool.tile([128, L, L], bf, name=f"bigT{u}", tag="bigT")
    tanhT = pool.tile([128, L, L], bf, name=f"tanhT{u}", tag="tanhT")
    add_engs = [nc.vector, nc.vector, nc.vector, nc.gpsimd]
    for h in range(NCH):
        isl = slice(h * CL, (h + 1) * CL)
        with W(0.02 + 0.002 * h):
            add_engs[h].tensor_tensor(
                bigT[:, isl, :],
                qTs[:, isl, :].broadcast_to([128, CL, L]),
                kTs[:].broadcast_to([128, CL, L]),
                op=mybir.AluOpType.add,
            )
        with W(0.021 + 0.002 * h):
            nc.scalar.activation(
                tanhT[:, isl, :].rearrange("p i j -> p (i j)"),
                bigT[:, isl, :].rearrange("p i j -> p (i j)"),
                mybir.ActivationFunctionType.Tanh)

    # ---- yall[i, r*256+h] = (x @ W_r)[i, h]: pipelined behind W_rel DMAs ----
    yall = pool.tile([L, NREL * H], bf, name=f"yall{u}", tag="yall")
    yps = []
    for g in range(4):
        yp = psum.tile([L, 512], f32, name=f"yp{g}{u}", tag=f"yp{g}")
        with W(0.012 + 0.002 * g):
            for t in range(2):
                nc.tensor.matmul(yp[:], apk[:, t, 0:48], wpk[g][:, t, :],
                                 start=(t == 0), stop=(t == 1))
        yps.append(yp)

    # ---- yall PSUM -> SBUF drains (split ACT/DVE) ----
    def psum_drain(eng, dst, src):
        if eng is nc.scalar:
            nc.scalar.copy(dst, src)
        else:
            eng.tensor_copy(dst, src)

    drain_engs = [nc.vector, nc.scalar, nc.vector, nc.vector]
    for g in range(4):
        with W(0.03 + 0.001 * g):
            psum_drain(drain_engs[g], yall[:, 512 * g:512 * (g + 1)], yps[g][:])

    # ---- scores[i, j] = v . tanhT[:, i, j]: one tiny N=1 matmul per column
    # (strided lhsT); writes the [48, 48] score grid straight into PSUM with
    # no cross-partition reshape needed.
    vvr = apk[:, 0, 304:305]
    sq_ps = psum.tile([L, L], f32, name=f"sq{u}", tag="sq_ps")
    with W(0.032):
        for j in range(L):
            nc.tensor.matmul(sq_ps[:, j:j + 1], tanhT[:, :, j], vvr,
                             start=True, stop=True)

    # ---- softmax + masked per-relation attention weights ----
    expS = pool.tile([L, 1, L], bf, name=f"expS{u}", tag="expS")
    rowsum = pool.tile([L, 1], f32, name=f"rs{u}", tag="rowsum")
    with W(0.034):
        nc.scalar.activation(expS[:, 0, :], sq_ps[:],
                             mybir.ActivationFunctionType.Exp,
                             accum_out=rowsum[:])
    recip = pool.tile([L, 1], f32, name=f"rc{u}", tag="recip")
    with W(0.035):
        nc.vector.reciprocal(recip[:], rowsum[:])
    maskt = spk[:].rearrange("i (r j) -> i r j", r=NREL)
    attnW = pool.tile([L, NREL, L], bf, name=f"attnW{u}", tag="attnW")
    NH = NREL // 2
    for a in range(2):
        rsl = slice(a * NH, (a + 1) * NH)
        with W(0.036 + 0.001 * a):
            nc.vector.scalar_tensor_tensor(
                attnW[:, rsl, :],
                expS[:].broadcast_to([L, NH, L]),
                recip[:],
                maskt[:, rsl, :],
                op0=mybir.AluOpType.mult,
                op1=mybir.AluOpType.mult,
            )

    # ---- h^T = W_root^T x^T + sum_r yall_r^T A_r + b_rgcn, in 2 H-halves ----
    hT_ps = [psum.tile([128, L], f32, name=f"hT{t}{u}", tag=("psA", "psB")[t])
             for t in range(2)]
    with W(0.031):
        for t in range(2):
            for tt in range(2):
                nc.tensor.matmul(hT_ps[t][:], wts[:, tt, 128 * t:128 * (t + 1)],
                                 apk[:, tt, 0:48], start=(tt == 0), stop=False)
    with W(0.038):
        for t in range(2):
            for r in range(NREL):
                nc.tensor.matmul(hT_ps[t][:],
                                 yall[:, H * r + 128 * t:H * r + 128 * (t + 1)],
                                 attnW[:, r, :], start=False, stop=(r == NREL - 1))
    hT = pool.tile([128, 2, L], bf, name=f"hTs{u}", tag="hTs")
    with W(0.04):
        for t in range(2):
            nc.vector.tensor_scalar_add(hT[:, t, :], hT_ps[t][:], bpk[:, t:t + 1])

    # ---- GraphConv (g-slice): out^T = W_self_c^T hT + (W_nbr_c^T s + b) ----
    sT = pool.tile([128, 2], bf, name=f"sT{u}", tag="sT")
    with W(0.041), nc.allow_low_precision(reason="48-term sum in bf16, |h|~1"):
        for t in range(2):
            nc.vector.reduce_sum(sT[:, t:t + 1], hT[:, t, :],
                                 axis=mybir.AxisListType.X)
    nb_ps = psum.tile([GS, 1], f32, name=f"nb{u}", tag="yp0")
    out_ps = psum.tile([GS, L], f32, name=f"op{u}", tag="yp1")
    with W(0.042):
        for t in range(2):
            nc.tensor.matmul(nb_ps[:], wts[:, t, 288:320], sT[:, t:t + 1],
                             start=(t == 0), stop=(t == 1))
        for t in range(2):
            nc.tensor.matmul(out_ps[:], wts[:, t, 256:288], hT[:, t, :],
                             start=(t == 0), stop=(t == 1))
    nbs = pool.tile([GS, 1], f32, name=f"nbs{u}", tag="nbs")
    with W(0.043):
        nc.vector.tensor_scalar_add(nbs[:], nb_ps[:], bpk[0:GS, 2:3])
    outs = pool.tile([GS, L], f32, name=f"outs{u}", tag="outs")
    with W(0.044):
        nc.vector.tensor_scalar_add(outs[:], out_ps[:], nbs)
    with W(0.045):
        nc.sync.dma_start(d["yout"].ap(), outs[:])


def build_program(n_cores=N_CORES, collective=False, repeat=1):
    """Build + schedule + compile the Bass program."""
    import concourse.bacc as bacc
    import concourse.mybir as mybir
    import concourse.tile as tile

    bf = mybir.dt.bfloat16
    f32 = mybir.dt.float32
    nc = bacc.Bacc("TRN2", debug=False, num_devices=n_cores)

    d = {}
    d["apack"] = nc.dram_tensor("apack", [128, 2, 305], bf, kind="ExternalInput")
    for g in range(4):
        d[f"wpk{g}"] = nc.dram_tensor(f"wpk{g}", [128, 2, 512], bf,
                                      kind="ExternalInput")
    d["wtpack"] = nc.dram_tensor("wtpack", [128, 2, 320], bf,
                                 kind="ExternalInput")
    d["spack"] = nc.dram_tensor("spack", [L, NREL * L], bf,
                                kind="ExternalInput")
    d["bpk"] = nc.dram_tensor("bpk", [128, 3], f32, kind="ExternalInput")
    d["yout"] = nc.dram_tensor("yout", [GS, L], f32, kind="ExternalOutput")

    with tile.TileContext(nc) as tc:
        with (
            tc.tile_pool(name="sbuf", bufs=1) as pool,
            tc.tile_pool(name="psum", bufs=1, space="PSUM") as psum,
            tc.tile_pool(name="dram", bufs=1, space="DRAM") as dram,
        ):
            for rep in range(repeat):
                _emit_body(nc, mybir, tc, pool, psum, dram, d, rep, n_cores)
                tc.tile_update_base_wait()

    nc.compile()
    return nc


def _prepare_in_maps(global_features, speaker, Wq, Wk, v, W_rel, W_root, b_rgcn,
                     W_nbr, W_self, b_gcn):
    """Host-side routing: pick the <=8 live relation slices, build masks, pack
    per-core shards (bf16 weights; each core gets the full RGCN weights and
    its 32-column slice of the GraphConv weights)."""
    import concourse.mybir as mybir
    bf = mybir.dt.np(mybir.dt.bfloat16)
    f32 = np.float32
    x = np.ascontiguousarray(global_features, dtype=f32)
    sp = np.asarray(speaker).astype(np.int64)
    n = L

    ii, jj = np.meshgrid(np.arange(n), np.arange(n), indexing="ij")
    direction = (ii >= jj).astype(np.int64)
    et = 2 * (sp[ii] * n + sp[jj]) + direction  # [48, 48] edge-type grid

    rel_ids = np.unique(et)
    assert len(rel_ids) <= NREL, f"{len(rel_ids)} live relations > {NREL}"
    masks = np.zeros((NREL, n, n), dtype=f32)
    rel_pad = np.full(NREL, rel_ids[0], dtype=np.int64)
    for s, rid in enumerate(rel_ids):
        masks[s] = (et == rid)
        rel_pad[s] = rid
    # padded slots keep zero masks -> contribute nothing

    W_used = np.asarray(W_rel)[rel_pad].astype(f32)       # [8, 256, 256]
    Wq = np.asarray(Wq, dtype=f32)
    Wk = np.asarray(Wk, dtype=f32)
    W_root = np.asarray(W_root, dtype=f32)
    W_self = np.asarray(W_self, dtype=f32)
    W_nbr = np.asarray(W_nbr, dtype=f32)
    b_rgcn = np.asarray(b_rgcn, dtype=f32)
    b_gcn = np.asarray(b_gcn, dtype=f32)
    vv = np.asarray(v, dtype=f32)

    # attention pack [128, 2, 305]: x^T halves, Wq, Wk, v
    apack = np.zeros((128, 2, 305), dtype=f32)
    xt = x.T.reshape(2, 128, n)                            # [t, p, i]
    for t in range(2):
        apack[:, t, 0:48] = xt[t]
        apack[:, t, 48:176] = Wq[128 * t:128 * (t + 1), :]
        apack[:, t, 176:304] = Wk[128 * t:128 * (t + 1), :]
    apack[:, 0, 304] = vv
    apack = apack.astype(bf)

    # W_rel rhs pack: wrel[d, r*256+h]; 4 groups of 512 cols, [p, t, 512]
    wrel = W_used.transpose(1, 0, 2).reshape(D, NREL * H)  # [d, r*H+h]
    wpks = []
    for g in range(4):
        cols = wrel[:, 512 * g:512 * (g + 1)].reshape(2, 128, 512)
        wpks.append(np.ascontiguousarray(cols.transpose(1, 0, 2)).astype(bf))

    maskw = np.ascontiguousarray(
        masks.transpose(1, 0, 2)).reshape(n, NREL * n).astype(bf)

    in_maps = []
    for c in range(N_CORES):
        sl = slice(c * GS, (c + 1) * GS)
        wtpack = np.zeros((128, 2, 320), dtype=f32)
        for t in range(2):
            wtpack[:, t, 0:256] = W_root[128 * t:128 * (t + 1), :]
            wtpack[:, t, 256:288] = W_self[128 * t:128 * (t + 1), sl]
            wtpack[:, t, 288:320] = W_nbr[128 * t:128 * (t + 1), sl]
        bpk = np.zeros((128, 3), dtype=f32)
        bpk[:, 0] = b_rgcn[0:128]
        bpk[:, 1] = b_rgcn[128:256]
        bpk[0:GS, 2] = b_gcn[sl]
        in_maps.append({
            "apack": apack,
            "wpk0": wpks[0], "wpk1": wpks[1],
            "wpk2": wpks[2], "wpk3": wpks[3],
            "wtpack": wtpack.astype(bf),
            "spack": maskw,
            "bpk": bpk,
        })
    return in_maps


def kernel(global_features, speaker, Wq, Wk, v, W_rel, W_root, b_rgcn,
           W_nbr, W_self, b_gcn):
    global _compiled
    from concourse.bass_utils import run_bass_kernel_spmd

    if _compiled is None:
        _compiled = build_program()
    nc = _compiled

    in_maps = _prepare_in_maps(global_features, speaker, Wq, Wk, v, W_rel,
                               W_root, b_rgcn, W_nbr, W_self, b_gcn)
    res = run_bass_kernel_spmd(nc, in_maps, core_ids=list(range(N_CORES)))
    outT = np.concatenate([res.results[c]["yout"] for c in range(N_CORES)], axis=0)
    return np.ascontiguousarray(outT.T)
